# revision 1
# baseline (speedup 1.0000x reference)
"""CriticGCN bass kernel: per-core program processing NG graphs.

Node space: N=50048 = 128*391 padded (real 50000). Digits: lo = n & 127
(partitions), hi = n >> 7 (free dim).

Phases per graph:
  deg:  histogram over col via one-hot matmuls accumulated in PSUM [128,391]
  y:    dinv = 1/sqrt(deg+1); y = dinv * (x @ conv_w)  (fp16 pairs -> DRAM)
  gath: y[row] via gpsimd ap_gather over 4 node ranges (replicated table)
  s:    scatter-add of y[row] into col slots via one-hot matmuls (PSUM)
  post: out = dinv*(s+y)+b; h=relu+x; MLP 2->8->8 leaky; masked sum pool
"""
import numpy as np
import concourse.bass as bass
import concourse.mybir as mybir
from concourse.bass import IndirectOffsetOnAxis, ts

Alu = mybir.AluOpType
Act = mybir.ActivationFunctionType

NLO, NHI = 128, 391
N = NLO * NHI            # 50048
NREAL = 50000
NRANGE = 4
RNG = N // NRANGE        # 12512 nodes per gather range (int16-safe idx)


def build_graph_program(tc, pool, spool, psp, consts, ei_d, x_d, ydram_d, res_d,
                        E, weights, SUP=10, CHT=125, PHASES=7):
    """Emit the program for one graph. ei_d: int32 [2, 2E] dram; x_d: [NREAL,2] f32;
    ydram_d: internal dram [N*2] fp16; res_d: [1,8] f32 slice of output."""
    nc = tc.nc
    f16, f32, i32, i16 = (mybir.dt.float16, mybir.dt.float32,
                          mybir.dt.int32, mybir.dt.int16)
    conv_w, conv_b, W1, b1, W2, b2 = weights
    iolo, iohisup, iohi2, maskt, onest = consts[:5]
    T = E // 128                      # edge tiles
    NSUP = T // SUP                   # deg/s loop iterations
    NCH = T // CHT                    # gather chunks
    assert T % SUP == 0 and T % CHT == 0

    ei32 = ei_d[:]                    # [2, 2E]
    row_pt = ei32[0, :].rearrange("(p t) -> p t", p=128)   # [128, 2T] int32 pairs
    col_pt = ei32[1, :].rearrange("(p t) -> p t", p=128)

    # ---------------- deg phase ----------------
    psum_deg = psp.tile([128, NHI], f32, space="PSUM", tag="psdeg")
    nc.vector.memset(psum_deg[:], 0.0)

    def digits_from_stream(src_pt, i, nsup_t, tag):
        """DMA + compact + digit-extract one super of nsup_t tiles.
        Returns (lo16, hi16) tiles [128, nsup_t]."""
        pairs = spool.tile([128, 2 * nsup_t], i32, tag="d_pairs")
        nc.sync.dma_start(pairs[:], src_pt[:, ts(i, 2 * nsup_t)])
        c32 = spool.tile([128, nsup_t], i32, tag="d_c32")
        nc.vector.tensor_copy(
            c32[:], pairs[:].rearrange("p (t two) -> p t two", two=2)[:, :, 0])
        lo32 = spool.tile([128, nsup_t], i32, tag="d_lo32")
        hi32 = spool.tile([128, nsup_t], i32, tag="d_hi32")
        nc.vector.tensor_scalar(lo32[:], c32[:], 127, None, op0=Alu.bitwise_and)
        nc.vector.tensor_scalar(hi32[:], c32[:], 7, None, op0=Alu.logical_shift_right)
        lo16 = spool.tile([128, nsup_t], f16, tag="d_lo16")
        hi16 = spool.tile([128, nsup_t], f16, tag="d_hi16")
        nc.vector.tensor_copy(lo16[:], lo32[:])
        nc.vector.tensor_copy(hi16[:], hi32[:])
        return lo16, hi16

    deg_loop = tc.For_i(0, NSUP) if (PHASES & 1) else tc.For_i(0, 1)
    with deg_loop as i:
        lo16, hi16 = digits_from_stream(col_pt, i, SUP, "dg")
        ohlo = spool.tile([128, SUP * 128], f16, tag="ohlo")
        ohhi = spool.tile([128, SUP * NHI], f16, tag="big")
        nc.vector.tensor_tensor(ohlo[:], iolo[:],
                                lo16[:].to_broadcast([128, SUP, 128]),
                                op=Alu.is_equal)
        nc.vector.tensor_tensor(ohhi[:], iohisup[:],
                                hi16[:].to_broadcast([128, SUP, NHI]),
                                op=Alu.is_equal)
        for u in range(SUP):
            nc.tensor.matmul(psum_deg[:],
                             lhsT=ohlo[:, u * 128:(u + 1) * 128],
                             rhs=ohhi[:, u * NHI:(u + 1) * NHI],
                             start=False, stop=False, skip_group_check=True)

    # ---------------- dinv / y ----------------
    deg_sb = pool.tile([128, NHI], f32, tag="deg")
    nc.vector.tensor_scalar(deg_sb[:], psum_deg[:], 1.0, None, op0=Alu.add)
    sq = pool.tile([128, NHI], f32, tag="sq")
    dinv = pool.tile([128, NHI], f32, tag="dinv")
    nc.scalar.activation(sq[:], deg_sb[:], Act.Sqrt)
    nc.vector.reciprocal(dinv[:], sq[:])

    # x digit layout [128, NHI, 2]
    xdig = pool.tile([128, NHI, 2], f32, tag="xdig")
    nc.vector.memset(xdig[:], 0.0)
    nc.sync.dma_start(xdig[:, :390, :],
                      x_d[0:49920, :].rearrange("(h l) c -> l h c", l=128))
    nc.sync.dma_start(xdig[0:80, 390, :], x_d[49920:50000, :])

    tmp = pool.tile([128, NHI], f32, tag="tmpa")
    xw0 = pool.tile([128, NHI], f32, tag="xw0")
    xw1 = pool.tile([128, NHI], f32, tag="xw1")
    nc.vector.tensor_scalar(tmp[:], xdig[:, :, 0], float(conv_w[0, 0]), None, op0=Alu.mult)
    nc.vector.scalar_tensor_tensor(xw0[:], xdig[:, :, 1], float(conv_w[1, 0]), tmp[:], op0=Alu.mult, op1=Alu.add)
    nc.vector.tensor_scalar(tmp[:], xdig[:, :, 0], float(conv_w[0, 1]), None, op0=Alu.mult)
    nc.vector.scalar_tensor_tensor(xw1[:], xdig[:, :, 1], float(conv_w[1, 1]), tmp[:], op0=Alu.mult, op1=Alu.add)
    y16 = pool.tile([128, NHI, 2], f16, tag="y16")
    nc.vector.tensor_tensor(y16[:, :, 0], xw0[:], dinv[:], op=Alu.mult)
    nc.vector.tensor_tensor(y16[:, :, 1], xw1[:], dinv[:], op=Alu.mult)
    # ydram[(hi*128+lo)*2 + c] = y16[lo, hi, c]
    nc.sync.dma_start(
        ydram_d[:].rearrange("(h l c) -> l h c", l=128, c=2), y16[:])

    # ---------------- gather phase ----------------
    # table = packed fp16 pairs viewed as f32; out-of-range rows redirect to a
    # zero slot (index RNG), so merging ranges is a pure bitwise-or.
    ydram32 = ydram_d[:].bitcast(f32)          # [N] packed pairs
    ygath = pool.tile([128, T], f32, tag="ygath")
    nc.vector.memset(ygath[:], 0.0)
    eyebits = consts[5]                        # [128, 16*CHT] int32 (p%16==k -> -1)
    for r in range(NRANGE):
        ytab = pool.tile([128, RNG + 2], f32, tag="ytab")
        nc.sync.dma_start(ytab[:, :RNG],
                          ydram32[r * RNG:(r + 1) * RNG].partition_broadcast(128))
        nc.vector.memset(ytab[:, RNG:], 0.0)
        with tc.For_i(0, NCH) as i:
            pairs = spool.tile([128, 2 * CHT], i32, tag="ga_pairs")
            nc.sync.dma_start(pairs[:], row_pt[:, ts(i, 2 * CHT)])
            r32 = spool.tile([128, CHT], i32, tag="ga_r32")
            nc.vector.tensor_copy(
                r32[:], pairs[:].rearrange("p (t two) -> p t two", two=2)[:, :, 0])
            # local idx with OOR -> RNG (zero slot)
            idx32 = spool.tile([128, CHT], i32, tag="ga_idx32")
            nc.vector.tensor_scalar(idx32[:], r32[:], -r * RNG, None, op0=Alu.add)
            nc.vector.tensor_scalar(idx32[:], idx32[:], -1, None, op0=Alu.max)
            nc.vector.tensor_scalar(idx32[:], idx32[:], RNG, None, op0=Alu.min)
            eqm = spool.tile([128, CHT], i32, tag="ga_eqm")
            nc.vector.tensor_scalar(eqm[:], idx32[:], -1, None, op0=Alu.is_equal)
            nc.vector.scalar_tensor_tensor(idx32[:], eqm[:], RNG + 1, idx32[:],
                                           op0=Alu.mult, op1=Alu.add)
            idx16 = spool.tile([128, CHT], i16, tag="ga_idx16")
            nc.vector.tensor_copy(idx16[:], idx32[:])
            gout = spool.tile([128, 16 * CHT], f32, tag="big")
            nc.gpsimd.ap_gather(gout[:], ytab[:], idx16[:], channels=128,
                                num_elems=RNG + 2, d=1, num_idxs=16 * CHT)
            # extract wrapped -> natural: nat[p, t] = gout[p, t*16 + p%16]
            # = or-reduce over k of gout[p, t*16+k] & eyebits[p, k]
            gob = gout[:].bitcast(i32)
            nc.vector.tensor_tensor(gob, gob, eyebits[:], op=Alu.bitwise_and)
            ext = spool.tile([128, CHT], i32, tag="ga_ext")
            nc.vector.tensor_reduce(
                ext[:], gob.rearrange("p (t k) -> p t k", k=16),
                axis=mybir.AxisListType.X, op=Alu.bitwise_or)
            nc.vector.tensor_tensor(ygath[:].bitcast(i32)[:, ts(i, CHT)],
                                    ygath[:].bitcast(i32)[:, ts(i, CHT)],
                                    ext[:], op=Alu.bitwise_or)

    # ---------------- s scatter phase ----------------
    psum_s0 = psp.tile([128, NHI], f32, space="PSUM", tag="pss0")
    psum_s1 = psp.tile([128, NHI], f32, space="PSUM", tag="pss1")
    nc.vector.memset(psum_s0[:], 0.0)
    nc.vector.memset(psum_s1[:], 0.0)
    ygath16 = ygath[:].bitcast(f16)   # [128, 2T]
    s_loop = tc.For_i(0, NSUP) if (PHASES & 4) else tc.For_i(0, 1)
    with s_loop as i:
        lo16, hi16 = digits_from_stream(col_pt, i, SUP, "sc")
        ohlo = spool.tile([128, SUP * 128], f16, tag="ohlo")
        nc.vector.tensor_tensor(ohlo[:], iolo[:],
                                lo16[:].to_broadcast([128, SUP, 128]),
                                op=Alu.is_equal)
        rhs = spool.tile([128, SUP, 2 * NHI], f16, tag="big")
        for u in range(SUP):
            nc.vector.scalar_tensor_tensor(
                rhs[:, u, :], iohi2[:], hi16[:, u:u + 1],
                ygath16[:, ts(i, 2 * SUP)].rearrange("p (t c) -> p t c", c=2)[:, u, :].to_broadcast([128, 2, NHI]),
                op0=Alu.is_equal, op1=Alu.mult)
        for u in range(SUP):
            nc.tensor.matmul(psum_s0[:], lhsT=ohlo[:, u * 128:(u + 1) * 128],
                             rhs=rhs[:, u, 0:NHI],
                             start=False, stop=False, skip_group_check=True)
            nc.tensor.matmul(psum_s1[:], lhsT=ohlo[:, u * 128:(u + 1) * 128],
                             rhs=rhs[:, u, NHI:2 * NHI],
                             start=False, stop=False, skip_group_check=True)

    # ---------------- post + MLP + pool ----------------
    h = pool.tile([128, NHI, 2], f32, tag="h")
    stot = pool.tile([128, NHI], f32, tag="stot")
    for ch, ps in ((0, psum_s0), (1, psum_s1)):
        nc.vector.tensor_tensor(stot[:], ps[:], y16[:, :, ch], op=Alu.add)
        nc.vector.tensor_tensor(stot[:], stot[:], dinv[:], op=Alu.mult)
        nc.vector.tensor_scalar(stot[:], stot[:], float(conv_b[ch]), None, op0=Alu.add)
        nc.vector.tensor_scalar(stot[:], stot[:], 0.0, None, op0=Alu.max)
        nc.vector.tensor_tensor(h[:, :, ch], stot[:], xdig[:, :, ch], op=Alu.add)

    h1 = pool.tile([128, 8, NHI], f32, tag="h1")
    tmp2 = pool.tile([128, NHI], f32, tag="tmpb")
    for j in range(8):
        nc.vector.tensor_scalar(tmp2[:], h[:, :, 0], float(W1[0, j]), None, op0=Alu.mult)
        nc.vector.scalar_tensor_tensor(h1[:, j, :], h[:, :, 1], float(W1[1, j]), tmp2[:], op0=Alu.mult, op1=Alu.add)
        nc.vector.tensor_scalar(h1[:, j, :], h1[:, j, :], float(b1[j]), None, op0=Alu.add)
        nc.vector.scalar_tensor_tensor(h1[:, j, :], h1[:, j, :], 0.01, h1[:, j, :], op0=Alu.mult, op1=Alu.max)

    h2r = pool.tile([128, 8], f32, tag="h2r")
    h2j = pool.tile([128, NHI], f32, tag="h2j")
    for j in range(8):
        nc.vector.tensor_scalar(h2j[:], h1[:, 0, :], float(W2[0, j]), None, op0=Alu.mult)
        for k in range(1, 8):
            nc.vector.scalar_tensor_tensor(h2j[:], h1[:, k, :], float(W2[k, j]), h2j[:], op0=Alu.mult, op1=Alu.add)
        nc.vector.tensor_scalar(h2j[:], h2j[:], float(b2[j]), None, op0=Alu.add)
        nc.vector.scalar_tensor_tensor(h2j[:], h2j[:], 0.01, h2j[:], op0=Alu.mult, op1=Alu.max)
        nc.vector.tensor_tensor(h2j[:], h2j[:], maskt[:], op=Alu.mult)
        nc.vector.tensor_reduce(h2r[:, j:j + 1], h2j[:], axis=mybir.AxisListType.X, op=Alu.add)

    psum_r = psp.tile([1, 8], f32, space="PSUM", tag="psr")
    nc.tensor.matmul(psum_r[:], lhsT=onest[:], rhs=h2r[:], start=True, stop=True)
    ressb = pool.tile([1, 8], f32, tag="res")
    nc.vector.tensor_copy(ressb[:], psum_r[:])
    nc.sync.dma_start(res_d[:], ressb[:])


def make_consts(SUP=10, CHT=125):
    io_lo = np.tile(np.arange(128, dtype=np.float16), (128, SUP))
    io_hi_sup = np.tile(np.arange(NHI, dtype=np.float16), (128, SUP))
    io_hi2 = np.tile(np.arange(NHI, dtype=np.float16), (128, 2))
    node_id = np.arange(N).reshape(NHI, NLO).T   # [lo, hi]
    mask = (node_id < NREAL).astype(np.float32)
    ones = np.ones((128, 1), np.float32)
    eye = np.zeros((128, 16), np.int32)
    eye[np.arange(128), np.arange(128) % 16] = -1
    eyebits = np.tile(eye, (1, CHT))
    return io_lo, io_hi_sup, io_hi2, mask, ones, eyebits


def build_core_program(nc, tc, NG, E, weights, SUP=10, CHT=125):
    """Declare IO and emit program for NG graphs. Returns input name list."""
    f16, f32, i32 = mybir.dt.float16, mybir.dt.float32, mybir.dt.int32
    import concourse.tile as tile
    ei_ds = [nc.dram_tensor(f"ei{g}", [2, 2 * E], i32, kind="ExternalInput").ap()
             for g in range(NG)]
    x_ds = [nc.dram_tensor(f"x{g}", [NREAL, 2], f32, kind="ExternalInput").ap()
            for g in range(NG)]
    iolo_d = nc.dram_tensor("iolo", [128, SUP * 128], f16, kind="ExternalInput").ap()
    iohisup_d = nc.dram_tensor("iohisup", [128, SUP * NHI], f16, kind="ExternalInput").ap()
    iohi2_d = nc.dram_tensor("iohi2", [128, 2 * NHI], f16, kind="ExternalInput").ap()
    mask_d = nc.dram_tensor("mask", [128, NHI], f32, kind="ExternalInput").ap()
    ones_d = nc.dram_tensor("ones", [128, 1], f32, kind="ExternalInput").ap()
    eyeb_d = nc.dram_tensor("eyebits", [128, 16 * CHT], i32, kind="ExternalInput").ap()
    res_d = nc.dram_tensor("res", [NG, 8], f32, kind="ExternalOutput").ap()
    ydram_d = nc.dram_tensor("ydram", [N * 2], f16, kind="Internal").ap()

    with tc.tile_pool(name="sb", bufs=1) as pool, \
         tc.tile_pool(name="sbs", bufs=2) as spool, \
         tc.tile_pool(name="ps", bufs=1, space="PSUM") as psp:
        iolo = pool.tile([128, SUP * 128], f16, tag="c_iolo")
        nc.sync.dma_start(iolo[:], iolo_d[:])
        iohisup = pool.tile([128, SUP * NHI], f16, tag="c_iohisup")
        nc.sync.dma_start(iohisup[:], iohisup_d[:])
        iohi2 = pool.tile([128, 2 * NHI], f16, tag="c_iohi2")
        nc.sync.dma_start(iohi2[:], iohi2_d[:])
        maskt = pool.tile([128, NHI], f32, tag="c_mask")
        nc.sync.dma_start(maskt[:], mask_d[:])
        onest = pool.tile([128, 1], f32, tag="c_ones")
        nc.sync.dma_start(onest[:], ones_d[:])
        eyebt = pool.tile([128, 16 * CHT], mybir.dt.int32, tag="c_eyeb")
        nc.sync.dma_start(eyebt[:], eyeb_d[:])
        consts = (iolo, iohisup, iohi2, maskt, onest, eyebt)
        for g in range(NG):
            import os
            build_graph_program(tc, pool, spool, psp, consts, ei_ds[g], x_ds[g],
                                ydram_d, res_d[g:g + 1, :], E, weights,
                                SUP=SUP, CHT=CHT,
                                PHASES=int(os.environ.get("PHASES", 7)))
    return [f"ei{g}" for g in range(NG)] + [f"x{g}" for g in range(NG)]


# ======================= public entry point =======================
import os as _os

_B, _E = 16, 1600000
_NCORES, _NG = 8, 2

def kernel(node_features, edge_index, conv_w, conv_b, lin1_w, lin1_b, lin2_w, lin2_b):
    """Full-input entry: shards 16 graphs as 2-per-core across 8 NeuronCores."""
    import sys
    if '/opt/trn_rl_repo' not in sys.path:
        sys.path.insert(0, '/opt/trn_rl_repo')
    import concourse.bacc as bacc
    import concourse.tile as tile
    from concourse.bass_utils import run_bass_kernel_spmd

    node_features = np.asarray(node_features, dtype=np.float32)
    edge_index = np.ascontiguousarray(np.asarray(edge_index, dtype=np.int64))
    weights = (np.asarray(conv_w, np.float32), np.asarray(conv_b, np.float32),
               np.asarray(lin1_w, np.float32), np.asarray(lin1_b, np.float32),
               np.asarray(lin2_w, np.float32), np.asarray(lin2_b, np.float32))

    SUP, CHT = 10, 125
    nc = bacc.Bacc("TRN2", target_bir_lowering=False, debug=False,
                   enable_asserts=False, num_devices=_NCORES)
    with tile.TileContext(nc) as tc:
        build_core_program(nc, tc, _NG, _E, weights, SUP=SUP, CHT=CHT)
    nc.compile()

    io_lo, io_hi_sup, io_hi2, mask, ones, eyebits = make_consts(SUP, CHT)
    ei32 = edge_index.view(np.int32)          # [16, 2, 2E]
    in_maps = []
    for c in range(_NCORES):
        m = {"iolo": io_lo, "iohisup": io_hi_sup, "iohi2": io_hi2,
             "mask": mask, "ones": ones, "eyebits": eyebits}
        for g in range(_NG):
            gi = c * _NG + g
            m[f"ei{g}"] = ei32[gi]
            m[f"x{g}"] = node_features[gi]
        in_maps.append(m)

    res = run_bass_kernel_spmd(nc, in_maps, core_ids=list(range(_NCORES)))
    out = np.zeros((_B, 8), np.float32)
    for c in range(_NCORES):
        out[c * _NG:(c + 1) * _NG] = res.results[c]["res"]
    return out



# revision 2
# speedup vs baseline: 73.6633x; 73.6633x over previous
"""CriticGCN bass kernel: per-core program processing NG graphs.

Node space: N=50048 = 128*391 padded (real 50000). Digits: lo = n & 127
(partitions), hi = n >> 7 (free dim).

Phases per graph:
  deg:  histogram over col via one-hot matmuls accumulated in PSUM [128,391]
  y:    dinv = 1/sqrt(deg+1); y = dinv * (x @ conv_w)  (fp16 pairs -> DRAM)
  gath: y[row] via gpsimd ap_gather over 4 node ranges (replicated table)
  s:    scatter-add of y[row] into col slots via one-hot matmuls (PSUM)
  post: out = dinv*(s+y)+b; h=relu+x; MLP 2->8->8 leaky; masked sum pool
"""
import numpy as np
import concourse.bass as bass
import concourse.mybir as mybir
from concourse.bass import IndirectOffsetOnAxis, ts

Alu = mybir.AluOpType
Act = mybir.ActivationFunctionType

NLO, NHI = 128, 391
N = NLO * NHI            # 50048
NREAL = 50000
NRANGE = 4
RNG = N // NRANGE        # 12512 nodes per gather range (int16-safe idx)


def build_graph_program(tc, pool, spool, psp, consts, ei_d, x_d, ydram_d, res_d,
                        E, weights, SUP=10, CHT=125, PHASES=7):
    """Emit the program for one graph. ei_d: int32 [2, 2E] dram; x_d: [NREAL,2] f32;
    ydram_d: internal dram [N*2] fp16; res_d: [1,8] f32 slice of output."""
    nc = tc.nc
    f16, f32, i32, i16 = (mybir.dt.float16, mybir.dt.float32,
                          mybir.dt.int32, mybir.dt.int16)
    conv_w, conv_b, W1, b1, W2, b2 = weights
    iolo, iohisup, iohi2, maskt, onest = consts[:5]
    T = E // 128                      # edge tiles
    NSUP = T // SUP                   # deg/s loop iterations
    NCH = T // CHT                    # gather chunks
    assert T % SUP == 0 and T % CHT == 0

    ei32 = ei_d[:]                    # [2, 2E]
    row_pt = ei32[0, :].rearrange("(p t) -> p t", p=128)   # [128, 2T] int32 pairs
    col_pt = ei32[1, :].rearrange("(p t) -> p t", p=128)

    # ---------------- deg phase ----------------
    psum_deg = psp.tile([128, NHI], f32, space="PSUM", tag="psdeg")
    nc.vector.memset(psum_deg[:], 0.0)

    def digits_from_stream(src_pt, i, nsup_t, tag):
        """DMA + compact + digit-extract one super of nsup_t tiles.
        Returns (lo16, hi16) tiles [128, nsup_t]."""
        pairs = spool.tile([128, 2 * nsup_t], i32, tag="d_pairs")
        nc.sync.dma_start(pairs[:], src_pt[:, ts(i, 2 * nsup_t)])
        c32 = spool.tile([128, nsup_t], i32, tag="d_c32")
        nc.vector.tensor_copy(
            c32[:], pairs[:].rearrange("p (t two) -> p t two", two=2)[:, :, 0])
        lo32 = spool.tile([128, nsup_t], i32, tag="d_lo32")
        hi32 = spool.tile([128, nsup_t], i32, tag="d_hi32")
        nc.vector.tensor_scalar(lo32[:], c32[:], 127, None, op0=Alu.bitwise_and)
        nc.vector.tensor_scalar(hi32[:], c32[:], 7, None, op0=Alu.logical_shift_right)
        lo16 = spool.tile([128, nsup_t], f16, tag="d_lo16")
        hi16 = spool.tile([128, nsup_t], f16, tag="d_hi16")
        nc.vector.tensor_copy(lo16[:], lo32[:])
        nc.vector.tensor_copy(hi16[:], hi32[:])
        return lo16, hi16

    deg_loop = tc.For_i(0, NSUP) if (PHASES & 1) else tc.For_i(0, 1)
    with deg_loop as i:
        lo16, hi16 = digits_from_stream(col_pt, i, SUP, "dg")
        ohlo = spool.tile([128, SUP * 128], f16, tag="ohlo")
        ohhi = spool.tile([128, SUP * NHI], f16, tag="big")
        nc.vector.tensor_tensor(ohlo[:], iolo[:],
                                lo16[:].to_broadcast([128, SUP, 128]),
                                op=Alu.is_equal)
        nc.vector.tensor_tensor(ohhi[:], iohisup[:],
                                hi16[:].to_broadcast([128, SUP, NHI]),
                                op=Alu.is_equal)
        for u in range(SUP):
            nc.tensor.matmul(psum_deg[:],
                             lhsT=ohlo[:, u * 128:(u + 1) * 128],
                             rhs=ohhi[:, u * NHI:(u + 1) * NHI],
                             start=False, stop=False, skip_group_check=True)

    # ---------------- dinv / y ----------------
    deg_sb = pool.tile([128, NHI], f32, tag="deg")
    nc.vector.tensor_scalar(deg_sb[:], psum_deg[:], 1.0, None, op0=Alu.add)
    sq = pool.tile([128, NHI], f32, tag="sq")
    dinv = pool.tile([128, NHI], f32, tag="dinv")
    nc.scalar.activation(sq[:], deg_sb[:], Act.Sqrt)
    nc.vector.reciprocal(dinv[:], sq[:])

    # x digit layout [128, NHI, 2]
    xdig = pool.tile([128, NHI, 2], f32, tag="xdig")
    nc.vector.memset(xdig[:], 0.0)
    nc.sync.dma_start(xdig[:, :390, :],
                      x_d[0:49920, :].rearrange("(h l) c -> l h c", l=128))
    nc.sync.dma_start(xdig[0:80, 390, :], x_d[49920:50000, :])

    tmp = pool.tile([128, NHI], f32, tag="tmpa")
    xw0 = pool.tile([128, NHI], f32, tag="xw0")
    xw1 = pool.tile([128, NHI], f32, tag="xw1")
    nc.vector.tensor_scalar(tmp[:], xdig[:, :, 0], float(conv_w[0, 0]), None, op0=Alu.mult)
    nc.vector.scalar_tensor_tensor(xw0[:], xdig[:, :, 1], float(conv_w[1, 0]), tmp[:], op0=Alu.mult, op1=Alu.add)
    nc.vector.tensor_scalar(tmp[:], xdig[:, :, 0], float(conv_w[0, 1]), None, op0=Alu.mult)
    nc.vector.scalar_tensor_tensor(xw1[:], xdig[:, :, 1], float(conv_w[1, 1]), tmp[:], op0=Alu.mult, op1=Alu.add)
    y16 = pool.tile([128, NHI, 2], f16, tag="y16")
    nc.vector.tensor_tensor(y16[:, :, 0], xw0[:], dinv[:], op=Alu.mult)
    nc.vector.tensor_tensor(y16[:, :, 1], xw1[:], dinv[:], op=Alu.mult)
    # ydram[(hi*128+lo)*2 + c] = y16[lo, hi, c]
    nc.sync.dma_start(
        ydram_d[:].rearrange("(h l c) -> l h c", l=128, c=2), y16[:])

    # ---------------- gather phase ----------------
    # table = packed fp16 pairs viewed as f32; out-of-range rows redirect to a
    # zero slot (index RNG), so merging ranges is a pure bitwise-or.
    ydram32 = ydram_d[:].bitcast(f32)          # [N] packed pairs
    ygath = pool.tile([128, T], f32, tag="ygath")
    nc.vector.memset(ygath[:], 0.0)
    eyebits = consts[5]                        # [128, 16*CHT] int32 (p%16==k -> -1)
    for r in range(NRANGE):
        ytab = pool.tile([128, RNG + 2], f32, tag="ytab")
        nc.sync.dma_start(ytab[:, :RNG],
                          ydram32[r * RNG:(r + 1) * RNG].partition_broadcast(128))
        nc.vector.memset(ytab[:, RNG:], 0.0)
        with tc.For_i(0, NCH) as i:
            pairs = spool.tile([128, 2 * CHT], i32, tag="ga_pairs")
            nc.sync.dma_start(pairs[:], row_pt[:, ts(i, 2 * CHT)])
            r32 = spool.tile([128, CHT], i32, tag="ga_r32")
            nc.vector.tensor_copy(
                r32[:], pairs[:].rearrange("p (t two) -> p t two", two=2)[:, :, 0])
            # local idx with OOR -> RNG (zero slot)
            idx32 = spool.tile([128, CHT], i32, tag="ga_idx32")
            nc.vector.tensor_scalar(idx32[:], r32[:], -r * RNG, None, op0=Alu.add)
            nc.vector.tensor_scalar(idx32[:], idx32[:], -1, None, op0=Alu.max)
            nc.vector.tensor_scalar(idx32[:], idx32[:], RNG, None, op0=Alu.min)
            eqm = spool.tile([128, CHT], i32, tag="ga_eqm")
            nc.vector.tensor_scalar(eqm[:], idx32[:], -1, None, op0=Alu.is_equal)
            nc.vector.scalar_tensor_tensor(idx32[:], eqm[:], RNG + 1, idx32[:],
                                           op0=Alu.mult, op1=Alu.add)
            idx16 = spool.tile([128, CHT], i16, tag="ga_idx16")
            nc.vector.tensor_copy(idx16[:], idx32[:])
            gout = spool.tile([128, 16 * CHT], f32, tag="big")
            nc.gpsimd.ap_gather(gout[:], ytab[:], idx16[:], channels=128,
                                num_elems=RNG + 2, d=1, num_idxs=16 * CHT)
            # extract wrapped -> natural: nat[p, t] = gout[p, t*16 + p%16]
            # = or-reduce over k of gout[p, t*16+k] & eyebits[p, k]
            gob = gout[:].bitcast(i32)
            nc.vector.tensor_tensor(gob, gob, eyebits[:], op=Alu.bitwise_and)
            ext = spool.tile([128, CHT], i32, tag="ga_ext")
            nc.vector.tensor_reduce(
                ext[:], gob.rearrange("p (t k) -> p t k", k=16),
                axis=mybir.AxisListType.X, op=Alu.bitwise_or)
            nc.vector.tensor_tensor(ygath[:].bitcast(i32)[:, ts(i, CHT)],
                                    ygath[:].bitcast(i32)[:, ts(i, CHT)],
                                    ext[:], op=Alu.bitwise_or)

    # ---------------- s scatter phase ----------------
    psum_s0 = psp.tile([128, NHI], f32, space="PSUM", tag="pss0")
    psum_s1 = psp.tile([128, NHI], f32, space="PSUM", tag="pss1")
    nc.vector.memset(psum_s0[:], 0.0)
    nc.vector.memset(psum_s1[:], 0.0)
    ygath16 = ygath[:].bitcast(f16)   # [128, 2T]
    s_loop = tc.For_i(0, NSUP) if (PHASES & 4) else tc.For_i(0, 1)
    with s_loop as i:
        lo16, hi16 = digits_from_stream(col_pt, i, SUP, "sc")
        ohlo = spool.tile([128, SUP * 128], f16, tag="ohlo")
        nc.vector.tensor_tensor(ohlo[:], iolo[:],
                                lo16[:].to_broadcast([128, SUP, 128]),
                                op=Alu.is_equal)
        rhs = spool.tile([128, SUP, 2 * NHI], f16, tag="big")
        for u in range(SUP):
            nc.vector.scalar_tensor_tensor(
                rhs[:, u, :], iohi2[:], hi16[:, u:u + 1],
                ygath16[:, ts(i, 2 * SUP)].rearrange("p (t c) -> p t c", c=2)[:, u, :].to_broadcast([128, 2, NHI]),
                op0=Alu.is_equal, op1=Alu.mult)
        for u in range(SUP):
            nc.tensor.matmul(psum_s0[:], lhsT=ohlo[:, u * 128:(u + 1) * 128],
                             rhs=rhs[:, u, 0:NHI],
                             start=False, stop=False, skip_group_check=True)
            nc.tensor.matmul(psum_s1[:], lhsT=ohlo[:, u * 128:(u + 1) * 128],
                             rhs=rhs[:, u, NHI:2 * NHI],
                             start=False, stop=False, skip_group_check=True)

    # ---------------- post + MLP + pool ----------------
    h = pool.tile([128, NHI, 2], f32, tag="h")
    stot = pool.tile([128, NHI], f32, tag="stot")
    for ch, ps in ((0, psum_s0), (1, psum_s1)):
        nc.vector.tensor_tensor(stot[:], ps[:], y16[:, :, ch], op=Alu.add)
        nc.vector.tensor_tensor(stot[:], stot[:], dinv[:], op=Alu.mult)
        nc.vector.tensor_scalar(stot[:], stot[:], float(conv_b[ch]), None, op0=Alu.add)
        nc.vector.tensor_scalar(stot[:], stot[:], 0.0, None, op0=Alu.max)
        nc.vector.tensor_tensor(h[:, :, ch], stot[:], xdig[:, :, ch], op=Alu.add)

    h1 = pool.tile([128, 8, NHI], f32, tag="h1")
    tmp2 = pool.tile([128, NHI], f32, tag="tmpb")
    for j in range(8):
        nc.vector.tensor_scalar(tmp2[:], h[:, :, 0], float(W1[0, j]), None, op0=Alu.mult)
        nc.vector.scalar_tensor_tensor(h1[:, j, :], h[:, :, 1], float(W1[1, j]), tmp2[:], op0=Alu.mult, op1=Alu.add)
        nc.vector.tensor_scalar(h1[:, j, :], h1[:, j, :], float(b1[j]), None, op0=Alu.add)
        nc.vector.scalar_tensor_tensor(h1[:, j, :], h1[:, j, :], 0.01, h1[:, j, :], op0=Alu.mult, op1=Alu.max)

    h2r = pool.tile([128, 8], f32, tag="h2r")
    h2j = pool.tile([128, NHI], f32, tag="h2j")
    for j in range(8):
        nc.vector.tensor_scalar(h2j[:], h1[:, 0, :], float(W2[0, j]), None, op0=Alu.mult)
        for k in range(1, 8):
            nc.vector.scalar_tensor_tensor(h2j[:], h1[:, k, :], float(W2[k, j]), h2j[:], op0=Alu.mult, op1=Alu.add)
        nc.vector.tensor_scalar(h2j[:], h2j[:], float(b2[j]), None, op0=Alu.add)
        nc.vector.scalar_tensor_tensor(h2j[:], h2j[:], 0.01, h2j[:], op0=Alu.mult, op1=Alu.max)
        nc.vector.tensor_tensor(h2j[:], h2j[:], maskt[:], op=Alu.mult)
        nc.vector.tensor_reduce(h2r[:, j:j + 1], h2j[:], axis=mybir.AxisListType.X, op=Alu.add)

    psum_r = psp.tile([1, 8], f32, space="PSUM", tag="psr")
    nc.tensor.matmul(psum_r[:], lhsT=onest[:], rhs=h2r[:], start=True, stop=True)
    ressb = pool.tile([1, 8], f32, tag="res")
    nc.vector.tensor_copy(ressb[:], psum_r[:])
    nc.sync.dma_start(res_d[:], ressb[:])


def make_consts(SUP=10, CHT=125):
    io_lo = np.tile(np.arange(128, dtype=np.float16), (128, SUP))
    io_hi_sup = np.tile(np.arange(NHI, dtype=np.float16), (128, SUP))
    io_hi2 = np.tile(np.arange(NHI, dtype=np.float16), (128, 2))
    node_id = np.arange(N).reshape(NHI, NLO).T   # [lo, hi]
    mask = (node_id < NREAL).astype(np.float32)
    ones = np.ones((128, 1), np.float32)
    eye = np.zeros((128, 16), np.int32)
    eye[np.arange(128), np.arange(128) % 16] = -1
    eyebits = np.tile(eye, (1, CHT))
    return io_lo, io_hi_sup, io_hi2, mask, ones, eyebits


def build_core_program(nc, tc, NG, E, weights, SUP=10, CHT=125):
    """Declare IO and emit program for NG graphs. Returns input name list."""
    f16, f32, i32 = mybir.dt.float16, mybir.dt.float32, mybir.dt.int32
    import concourse.tile as tile
    ei_ds = [nc.dram_tensor(f"ei{g}", [2, 2 * E], i32, kind="ExternalInput").ap()
             for g in range(NG)]
    x_ds = [nc.dram_tensor(f"x{g}", [NREAL, 2], f32, kind="ExternalInput").ap()
            for g in range(NG)]
    iolo_d = nc.dram_tensor("iolo", [128, SUP * 128], f16, kind="ExternalInput").ap()
    iohisup_d = nc.dram_tensor("iohisup", [128, SUP * NHI], f16, kind="ExternalInput").ap()
    iohi2_d = nc.dram_tensor("iohi2", [128, 2 * NHI], f16, kind="ExternalInput").ap()
    mask_d = nc.dram_tensor("mask", [128, NHI], f32, kind="ExternalInput").ap()
    ones_d = nc.dram_tensor("ones", [128, 1], f32, kind="ExternalInput").ap()
    eyeb_d = nc.dram_tensor("eyebits", [128, 16 * CHT], i32, kind="ExternalInput").ap()
    res_d = nc.dram_tensor("res", [NG, 8], f32, kind="ExternalOutput").ap()
    ydram_d = nc.dram_tensor("ydram", [N * 2], f16, kind="Internal").ap()

    with tc.tile_pool(name="sb", bufs=1) as pool, \
         tc.tile_pool(name="sbs", bufs=2) as spool, \
         tc.tile_pool(name="ps", bufs=1, space="PSUM") as psp:
        iolo = pool.tile([128, SUP * 128], f16, tag="c_iolo")
        nc.sync.dma_start(iolo[:], iolo_d[:])
        iohisup = pool.tile([128, SUP * NHI], f16, tag="c_iohisup")
        nc.sync.dma_start(iohisup[:], iohisup_d[:])
        iohi2 = pool.tile([128, 2 * NHI], f16, tag="c_iohi2")
        nc.sync.dma_start(iohi2[:], iohi2_d[:])
        maskt = pool.tile([128, NHI], f32, tag="c_mask")
        nc.sync.dma_start(maskt[:], mask_d[:])
        onest = pool.tile([128, 1], f32, tag="c_ones")
        nc.sync.dma_start(onest[:], ones_d[:])
        eyebt = pool.tile([128, 16 * CHT], mybir.dt.int32, tag="c_eyeb")
        nc.sync.dma_start(eyebt[:], eyeb_d[:])
        consts = (iolo, iohisup, iohi2, maskt, onest, eyebt)
        for g in range(NG):
            import os
            build_graph_program(tc, pool, spool, psp, consts, ei_ds[g], x_ds[g],
                                ydram_d, res_d[g:g + 1, :], E, weights,
                                SUP=SUP, CHT=CHT,
                                PHASES=int(os.environ.get("PHASES", 7)))
    return [f"ei{g}" for g in range(NG)] + [f"x{g}" for g in range(NG)]


# ======================= public entry point =======================
import os as _os

_B, _E = 16, 1600000
_NCORES, _NG = 8, 2

def build_and_inputs(node_features, edge_index, weights, SUP=10, CHT=125):
    """Build+compile the Bass program and the per-core input maps."""
    import sys
    if '/opt/trn_rl_repo' not in sys.path:
        sys.path.insert(0, '/opt/trn_rl_repo')
    import concourse.bacc as bacc
    import concourse.tile as tile

    nc = bacc.Bacc("TRN2", target_bir_lowering=False, debug=False,
                   enable_asserts=False, num_devices=_NCORES)
    with tile.TileContext(nc) as tc:
        build_core_program(nc, tc, _NG, _E, weights, SUP=SUP, CHT=CHT)
    nc.compile()

    io_lo, io_hi_sup, io_hi2, mask, ones, eyebits = make_consts(SUP, CHT)
    ei32 = edge_index.view(np.int32)          # [16, 2, 2E]
    in_maps = []
    for c in range(_NCORES):
        m = {"iolo": io_lo, "iohisup": io_hi_sup, "iohi2": io_hi2,
             "mask": mask, "ones": ones, "eyebits": eyebits}
        for g in range(_NG):
            gi = c * _NG + g
            m[f"ei{g}"] = ei32[gi]
            m[f"x{g}"] = node_features[gi]
        in_maps.append(m)
    return nc, in_maps


def kernel(node_features, edge_index, conv_w, conv_b, lin1_w, lin1_b, lin2_w, lin2_b):
    """Full-input entry: shards 16 graphs as 2-per-core across 8 NeuronCores."""
    import sys
    if '/opt/trn_rl_repo' not in sys.path:
        sys.path.insert(0, '/opt/trn_rl_repo')
    from concourse.bass_utils import run_bass_kernel_spmd

    node_features = np.asarray(node_features, dtype=np.float32)
    edge_index = np.ascontiguousarray(np.asarray(edge_index, dtype=np.int64))
    weights = (np.asarray(conv_w, np.float32), np.asarray(conv_b, np.float32),
               np.asarray(lin1_w, np.float32), np.asarray(lin1_b, np.float32),
               np.asarray(lin2_w, np.float32), np.asarray(lin2_b, np.float32))

    nc, in_maps = build_and_inputs(node_features, edge_index, weights)

    res = run_bass_kernel_spmd(nc, in_maps, core_ids=list(range(_NCORES)))
    out = np.zeros((_B, 8), np.float32)
    for c in range(_NCORES):
        out[c * _NG:(c + 1) * _NG] = res.results[c]["res"]
    return out

